# revision 1
# baseline (speedup 1.0000x reference)
"""Multi-head attention (embed 1024, 16 heads x 64) on 8 TRN2 NeuronCores.

Sharding: tensor-parallel over heads — each core owns 2 heads end-to-end
(qkv projection columns + attention), then per-(batch, head) AllToAlls
redistribute the per-head attention outputs so each core computes the
out-projection for its 256-token slice of each batch.

Compute is bf16 on the TensorEngine (fp32 PSUM accumulation). The engines
execute statically-ordered instruction streams, so emission order is
software-pipelined: projection chunks and out-projection slices are woven
INSIDE attention iterations (each iteration has ~5us of ScalarE-bound PE
slack that absorbs them; a block between iterations stalls the EXP stream
for its whole duration), and each iteration's softmax normalization
(reciprocal + broadcast + multiply) is deferred by one slot so the slow
one-partition reciprocal never blocks the PE or DVE streams.

Tail discipline: everything gated on a PEER (gathers, batch-1 out-proj)
is emitted strictly after the last A2A trigger — under cross-core skew
(observed up to ~16us between SPMD ranks) a peer-gated DMA emitted
earlier stalls the local queues and compounds the skew. Weight matrices
arrive host-pre-transposed so weight DMAs read 2KB-contiguous lines.

Layout:
  - host passes x TRANSPOSED [e, t] so projections contract e on partitions
    with no on-device transpose; projections produce Q/K/V as [head_dim, t].
  - scores are computed transposed: St[tk, tq] = Kt.T @ Qt, so the softmax
    sum over tk comes free from a ones-column appended to V: the PV matmul
    outputs [65, tq] with row 64 = sum of exp.
  - normalization: numerator+denominator copied to SBUF (frees the PV PSUM
    accumulator), then reciprocal + K=1 broadcast matmul + vector multiply.
  - out_proj consumes the AllToAll output directly (head-dim on partitions)
    and the result is PE-transposed back to row-major before the output DMA.
"""

import numpy as np
import ml_dtypes

import concourse.bass as bass
import concourse.tile as tile
from concourse import bacc, mybir
from concourse.bass_utils import run_bass_kernel_spmd
from concourse.masks import make_identity

N_CORES = 8
B, S, D = 2, 2048, 1024
T = B * S              # 4096 flattened tokens
HEADS = 16
DH = 64                # head dim
HPC = HEADS // N_CORES  # heads per core = 2
CW = HPC * DH          # per-core qkv width = 128
SCALE = DH ** -0.5
TC = T // N_CORES      # per-core output rows = 512 (256 per batch)
NW = TC // 2           # tokens per batch slice = 256
ET = D // 128          # e partition tiles = 8
F32 = mybir.dt.float32
BF16 = mybir.dt.bfloat16
EXP = mybir.ActivationFunctionType.Exp
BF = ml_dtypes.bfloat16

_CACHED_NC = None


def build():
    nc = bacc.Bacc(
        "TRN2",
        target_bir_lowering=False,
        debug=False,
        num_devices=N_CORES,
    )
    xt_ap = nc.dram_tensor("xt", [D, T], BF16, kind="ExternalInput").ap()
    # w_qkv slices arrive host-transposed as [p, et*c] so the weight DMA
    # reads 2KB-contiguous lines per partition (the on-device rearrange
    # had 256B runs: 1024 descriptors and ~1.1us of descriptor-gen each)
    wq_ap = nc.dram_tensor("wq", [128, ET * CW], BF16, kind="ExternalInput").ap()
    wk_ap = nc.dram_tensor("wk", [128, ET * CW], BF16, kind="ExternalInput").ap()
    wv_ap = nc.dram_tensor("wv", [128, ET * CW], BF16, kind="ExternalInput").ap()
    bq_ap = nc.dram_tensor("bq", [CW, 1], F32, kind="ExternalInput").ap()
    bk_ap = nc.dram_tensor("bk", [CW, 1], F32, kind="ExternalInput").ap()
    bv_ap = nc.dram_tensor("bv", [CW, 1], F32, kind="ExternalInput").ap()
    wout_ap = nc.dram_tensor("wout", [D, D], BF16, kind="ExternalInput").ap()
    bout_ap = nc.dram_tensor("bout", [128, ET], F32, kind="ExternalInput").ap()
    out_ap = nc.dram_tensor("out", [D, TC], F32, kind="ExternalOutput").ap()

    with tile.TileContext(nc) as tc:
        with (
            tc.tile_pool(name="singles", bufs=1) as singles,
            tc.tile_pool(name="xt", bufs=32) as xt_pool,
            tc.tile_pool(name="vt", bufs=2) as vt_pool,
            tc.tile_pool(name="exp", bufs=6) as exp_pool,
            tc.tile_pool(name="fo", bufs=2) as fo_pool,
            tc.tile_pool(name="small", bufs=2) as small_pool,
            tc.tile_pool(name="saved", bufs=8) as saved_pool,
            tc.tile_pool(name="mmps", bufs=2, space="PSUM") as mmps,
            tc.tile_pool(name="stps", bufs=2, space="PSUM") as stps,
            tc.tile_pool(name="pvps", bufs=1, space="PSUM") as pvps,
            tc.tile_pool(name="dram", bufs=1, space="DRAM") as dram,
        ):
            # A2A bounce buffers, one pair per (batch, head): shard j holds
            # tokens [j*256,(j+1)*256) of batch b, 64 head-dims per shard.
            dummy_in = dram.tile([N_CORES, 16], BF16, name="dummy_in")
            dummy_out = dram.tile([N_CORES, 16], BF16, name="dummy_out")
            a2a_in = [
                [dram.tile([N_CORES * DH, NW], BF16, name=f"a2a_in{b}_{h}")
                 for h in range(HPC)] for b in range(2)
            ]
            a2a_out = [
                [dram.tile([N_CORES * DH, NW], BF16, name=f"a2a_out{b}_{h}")
                 for h in range(HPC)] for b in range(2)
            ]

            # ---- constants / weights resident in SBUF ----
            # identity first: it has no DMA dependency, so the PE warm-up
            # matmuls below can start while the input DMAs stream
            identb = singles.tile([128, 128], BF16)
            make_identity(nc, identb)
            # ramp the PE p-state while the first x chunk + weights are in
            # flight: the projections otherwise start at 0.65-1.2GHz and
            # burn ~7us extra before the clock reaches full speed
            for _ in range(24):
                warm = mmps.tile([128, 128], F32, tag="mm", name="warm0")
                nc.tensor.matmul(warm, identb, identb)
            w_sb, b_sb = {}, {}
            for name, wap, bap in (
                ("q", wq_ap, bq_ap), ("k", wk_ap, bk_ap), ("v", wv_ap, bv_ap)
            ):
                w_sb[name] = singles.tile(
                    [128, ET, CW], BF16, tag=f"w{name}", name=f"w{name}_sb"
                )
                nc.gpsimd.dma_start(
                    out=w_sb[name],
                    in_=wap.rearrange("p (et c) -> p et c", et=ET),
                )
                b_sb[name] = singles.tile(
                    [CW, 1], F32, tag=f"b{name}", name=f"b{name}_sb"
                )
                nc.gpsimd.dma_start(out=b_sb[name], in_=bap)
            bout_sb = singles.tile([128, ET], F32)
            nc.gpsimd.dma_start(out=bout_sb, in_=bout_ap)
            ones64 = singles.tile([1, DH], BF16)
            nc.vector.memset(ones64, 1.0)
            # dummy collective: absorbs the collective-stream entry barrier +
            # first-trigger latency while the PE ramps. Emitted after the
            # preamble DMAs so the trigger's barrier wait doesn't stall them.
            nc.gpsimd.collective_compute(
                "AllToAll",
                mybir.AluOpType.bypass,
                replica_groups=[list(range(N_CORES))],
                ins=[dummy_in[:, :].opt()],
                outs=[dummy_out[:, :].opt()],
            )
            wout_sb = singles.tile([128, ET, D], BF16, tag="wout")

            # persistent activations
            qt = singles.tile([CW, T], BF16, tag="qt")   # [2h*64, t] transposed Q
            kt = singles.tile([CW, T], BF16, tag="kt")
            # partition-swapped copy of kt: alternating score matmuls load
            # weights into the OTHER PE-array row group, so each LDWEIGHTS
            # pulls ahead of the in-flight matmul instead of serializing
            kt2 = singles.tile([CW, T], BF16, tag="kt2")
            qt2 = singles.tile([CW, T], BF16, tag="qt2")
            # V natural per head, 65-wide tk-tiles (col 64 = ones for denom)
            vsb = [
                singles.tile(
                    [128, T // 128, DH + 1], BF16, tag=f"v{h}", name=f"v{h}_sb"
                )
                for h in range(HPC)
            ]
            for h in range(HPC):
                nc.vector.memset(vsb[h][:, :, DH:DH + 1], 1.0)
            # gathered head-features for this core's token rows, per batch
            g_sb = [
                singles.tile([128, ET, NW], BF16, tag=f"g{b}", name=f"g{b}_sb")
                for b in range(2)
            ]

            xt_view = xt_ap.rearrange("(et p) t -> p et t", p=128)

            chunk1024 = {}

            def emit_chunk_dma(tch2, spread=False):
                # 1024-token transfers: the x DMAs are DESCRIPTOR-RATE
                # bound (~42ns per per-partition run), so 1KB runs cap a
                # queue at ~24GB/s — 2KB runs double it. spread=True issues
                # the per-et DMAs from otherwise-idle sequencers (preamble
                # only): the sync sequencer takes ~600ns of descriptor-gen
                # per dma_start.
                engs = (
                    [nc.sync, nc.sync, nc.sync, nc.scalar,
                     nc.scalar, nc.scalar, nc.gpsimd, nc.sync]
                    if spread else [nc.sync] * ET
                )
                xs = []
                for et in range(ET):
                    xe = xt_pool.tile([128, 1024], BF16, tag="xt", name="xt_e")
                    # two half-partition DMAs per et tile: a DMA queue
                    # retires ~one 2KB descriptor per 42ns, so one [128,1024]
                    # transfer pins a queue for ~5.4us — the split doubles
                    # queue parallelism and halves the landing time
                    for ph in range(2):
                        engs[et].dma_start(
                            out=xe[ph * 64:(ph + 1) * 64, :],
                            in_=xt_view[ph * 64:(ph + 1) * 64, et,
                                        tch2 * 1024:(tch2 + 1) * 1024],
                        )
                    xs.append(xe)
                chunk1024[tch2] = xs

            def chunk_view(tch, et):
                """512-token projection view into the 1024-token tiles."""
                half = tch % 2
                return chunk1024[tch // 2][et][:, half * 512:(half + 1) * 512]


            def emit_proj_halves(tch, name):
                """One projection (q/k/v) of a 512-token chunk, split into
                two filler closures (4 accumulating matmuls each, PSUM
                accumulator carried across). The scores->EXP pipeline has
                only a 2-tile lookahead (stps double buffer), so a full
                ~1.8us projection piece woven between score tiles drains it
                and stalls ScalarE ~1us; half-pieces keep the bubble short."""
                st = {}

                def half_a():
                    st["pp"] = mmps.tile(
                        [CW, 512], F32, tag="mm", name="pp_proj"
                    )
                    for et in range(ET // 2):
                        nc.tensor.matmul(
                            st["pp"],
                            w_sb[name][:, et, :],
                            chunk_view(tch, et),
                            start=(et == 0),
                            stop=False,
                        )

                def half_b():
                    _finish_proj(tch, name, st["pp"])

                return half_a, half_b

            def emit_proj(tch, name):
                a, b = emit_proj_halves(tch, name)
                a()
                b()

            def _finish_proj(tch, name, pp):
                dest = {"q": qt, "k": kt, "v": None}[name]
                for et in range(ET // 2, ET):
                    nc.tensor.matmul(
                        pp,
                        w_sb[name][:, et, :],
                        chunk_view(tch, et),
                        start=False,
                        stop=(et == ET - 1),
                    )
                if dest is not None:
                    nc.vector.tensor_scalar_add(
                        dest[:, tch * 512:(tch + 1) * 512], pp, b_sb[name]
                    )
                    src_t, dst_t = (kt, kt2) if name == "k" else (qt, qt2)
                    sl = slice(tch * 512, (tch + 1) * 512)
                    # first-chunk swaps dodge the gpsimd queue: the dummy
                    # collective's entry-barrier wait sits there and would
                    # delay qt2/kt2 (gating the first score tiles). ScalarE's
                    # queue is free until the first EXP fires.
                    eng = nc.scalar if tch < 2 else nc.gpsimd
                    eng.dma_start(
                        out=dst_t[DH:2 * DH, sl], in_=src_t[0:DH, sl]
                    )
                    eng.dma_start(
                        out=dst_t[0:DH, sl], in_=src_t[DH:2 * DH, sl]
                    )
                else:
                    vt_tmp = vt_pool.tile([CW, 512], BF16, name="vt_tmp")
                    nc.vector.tensor_scalar_add(vt_tmp, pp, b_sb[name])
                    for tt in range(4):
                        ps2 = mmps.tile([128, 128], BF16, tag="mm", name="ps_vtr")
                        nc.tensor.transpose(
                            ps2, vt_tmp[:, tt * 128:(tt + 1) * 128], identb
                        )
                        ttg = tch * 4 + tt
                        for h in range(HPC):
                            nc.vector.tensor_copy(
                                vsb[h][:, ttg, 0:DH],
                                ps2[:, h * DH:(h + 1) * DH],
                            )

            def emit_projs(tch):
                for name in ("q", "k", "v"):
                    emit_proj(tch, name)

            def weave(b, h, tqh, fillers, scalar_copy=False):
                """Attention iteration with filler pieces spread between
                tk-tile groups so ScalarE's exp stream never starves while
                the PE works through a filler."""
                A = AttIter(b, h, tqh, scalar_copy=scalar_copy)
                k = len(fillers)
                for i, f in enumerate(fillers):
                    A.advance((i + 1) * 16 // (k + 1))
                    f()
                return A.finish()

            class AttIter:
                """Resumable attention iteration: 1024 queries of head h,
                batch b. advance(tk_hi) emits score/exp/PV work for tk-tiles
                up to tk_hi (PV pipelined one tile behind scores); finish()
                drains and returns (pvc, recip) for deferred normalization.
                Splitting lets the first iteration start as soon as the
                chunks covering its early tk-tiles are projected."""

                def __init__(self, b, h, tqh, scalar_copy=False):
                    self.b, self.h = b, h
                    self.po = h * DH
                    self.po2 = DH - self.po
                    self.tq0 = b * S + tqh * 1024
                    self.pv = pvps.tile([DH + 1, 1024], F32, name="pv")
                    self.exs = {}
                    self.sc_done = 0
                    self.pv_done = 0
                    self.scalar_copy = scalar_copy

                def _scores(self, tkt):
                    st = stps.tile([128, 1024], F32, tag="st", name="st")
                    k0 = self.b * S + tkt * 128
                    for nh in range(2):
                        if nh == 0:
                            lhsT = kt[self.po:self.po + DH, k0:k0 + 128]
                            rhs_q = qt[self.po:self.po + DH,
                                       self.tq0 + nh * 512:
                                       self.tq0 + (nh + 1) * 512]
                        else:
                            lhsT = kt2[self.po2:self.po2 + DH, k0:k0 + 128]
                            rhs_q = qt2[self.po2:self.po2 + DH,
                                        self.tq0 + nh * 512:
                                        self.tq0 + (nh + 1) * 512]
                        nc.tensor.matmul(
                            st[:, nh * 512:(nh + 1) * 512], lhsT, rhs_q
                        )
                    ex = exp_pool.tile([128, 1024], BF16, name="ex")
                    nc.scalar.activation(ex, st, EXP)
                    self.exs[tkt] = ex

                def _pvacc_pair(self, tkt):
                    # two tk-tiles per PV group, ordered so consecutive
                    # matmuls hit the same psum bank (N=1024 in one matmul
                    # fails the ISA check: out free size is capped at 512)
                    ex0, ex1 = self.exs.pop(tkt), self.exs.pop(tkt + 1)
                    for nh in range(2):
                        for tt, ex in ((tkt, ex0), (tkt + 1, ex1)):
                            nc.tensor.matmul(
                                self.pv[:, nh * 512:(nh + 1) * 512],
                                vsb[self.h][:, self.b * 16 + tt, :],
                                ex[:, nh * 512:(nh + 1) * 512],
                                start=(tt == 0),
                                stop=(tt == 15),
                            )

                def advance(self, tk_hi):
                    while self.sc_done < tk_hi:
                        self._scores(self.sc_done)
                        self.sc_done += 1
                        # PV trails the scores by one extra tile so its
                        # first matmul consumes an EXP that finished >=2
                        # tiles ago — it never stalls on a fresh ScalarE
                        # completion (PV slices showed ~+60-90ns waits).
                        # One MORE tile of lag regresses: the end-of-
                        # iteration drain then serializes two whole PV
                        # pairs behind the final EXPs (~+13us measured).
                        if self.pv_done + 1 < self.sc_done - 2:
                            self._pvacc_pair(self.pv_done)
                            self.pv_done += 2

                def finish(self):
                    self.advance(16)
                    while self.pv_done < 16:
                        self._pvacc_pair(self.pv_done)
                        self.pv_done += 2
                    # numerator + denominator leave PSUM in ONE bf16 copy
                    # (a separate [1,1024] denominator copy is lane-serial
                    # on the DVE: ~1.2us). In the tail the copy goes to the
                    # idle ScalarE so the DVE is free for what follows. The
                    # denominator row is then DMA-spread across 128
                    # partitions so the reciprocal runs 128-wide (a 1-lane
                    # [1,1024] reciprocal costs 6.5us on the DVE).
                    pvc = fo_pool.tile(
                        [DH + 1, 1024], BF16, tag="pvc", name="pvc"
                    )
                    if self.scalar_copy:
                        nc.scalar.activation(
                            pvc, self.pv, mybir.ActivationFunctionType.Copy
                        )
                    else:
                        nc.vector.tensor_copy(pvc, self.pv)
                    # both spread hops split across two queues (gpsimd +
                    # sync): a [1,1024]<->[128,8] DMA is 128 tiny runs on
                    # one queue (~1.5us); the halves drain in parallel
                    dn128 = small_pool.tile(
                        [128, 8], BF16, tag="dn128", name="dn128"
                    )
                    nc.gpsimd.dma_start(
                        out=dn128[0:64, :], in_=pvc[DH:DH + 1, 0:512]
                    )
                    nc.sync.dma_start(
                        out=dn128[64:128, :], in_=pvc[DH:DH + 1, 512:1024]
                    )
                    r128 = small_pool.tile([128, 8], BF16, tag="r128", name="r128")
                    with nc.allow_low_precision(
                        reason="softmax denom reciprocal feeds bf16 bcast"
                    ):
                        nc.vector.reciprocal(r128, dn128)
                    recip = small_pool.tile([1, 1024], BF16, name="recip")
                    nc.gpsimd.dma_start(
                        out=recip[:, 0:512], in_=r128[0:64, :]
                    )
                    nc.sync.dma_start(
                        out=recip[:, 512:1024], in_=r128[64:128, :]
                    )
                    return pvc, recip

            def emit_attention(b, h, tqh):
                return AttIter(b, h, tqh).finish()

            def emit_finish(b, h, tqh, pvc, recip):
                """Deferred normalization + scatter into the A2A input."""
                fo = fo_pool.tile([DH, 1024], BF16, name="fo")
                # head h of batch b goes to a2a_in[b][h]: shard j = rank j's
                # 256 tokens, 64 rows each. Issued from the gpsimd queue so
                # the collective trigger (also gpsimd) isn't stuck behind
                # out-DMAs queued on the sync engine; scattered per nh half
                # so the first DMA overlaps the second half's multiply.
                a2a_view = a2a_in[b][h][:, :].rearrange("(j p) t -> p j t", p=DH)
                j0 = tqh * 4
                for nh in range(2):
                    bc = mmps.tile([DH, 512], F32, tag="mm", name="bc")
                    nc.tensor.matmul(
                        bc, ones64, recip[:, nh * 512:(nh + 1) * 512]
                    )
                    nc.vector.tensor_mul(
                        fo[:, nh * 512:(nh + 1) * 512],
                        pvc[0:DH, nh * 512:(nh + 1) * 512],
                        bc,
                    )
                    # per-shard queue split: this scatter gates the A2A
                    # trigger directly
                    fo_j = fo[:, nh * 512:(nh + 1) * 512].rearrange(
                        "p (j t) -> p j t", j=2
                    )
                    for j, eng in ((0, nc.gpsimd), (1, nc.sync)):
                        eng.dma_start(
                            out=a2a_view[:, j0 + 2 * nh + j:
                                         j0 + 2 * nh + j + 1, :],
                            in_=fo_j[:, j:j + 1, :],
                        )

            def emit_a2a(b, h):
                nc.gpsimd.collective_compute(
                    "AllToAll",
                    mybir.AluOpType.bypass,
                    replica_groups=[list(range(N_CORES))],
                    ins=[a2a_in[b][h][:, :].opt()],
                    outs=[a2a_out[b][h][:, :].opt()],
                )

            def emit_gather(b, h, split=False):
                # rank r's shard lands at rows r*64..(r+1)*64 → head h's dims
                # are partitions h*64..h*64+64 of g_sb[b][:, r, :]. Emitted
                # just before the first consumer: this DMA waits on the
                # collective, and the sync engine issues triggers in order —
                # an early emission would stall every later DMA behind it.
                src = a2a_out[b][h][:, :].rearrange("(r p) t -> p r t", p=DH)
                if not split:
                    nc.sync.dma_start(
                        out=g_sb[b][h * DH:(h + 1) * DH, :, :], in_=src
                    )
                else:
                    # tail gathers sit on the mesh-end -> out-proj critical
                    # path: halve their drain time across two queues
                    # (ScalarE's queue is idle once the EXPs are done)
                    for rh, eng in ((0, nc.sync), (1, nc.scalar)):
                        eng.dma_start(
                            out=g_sb[b][h * DH:(h + 1) * DH,
                                        rh * 4:(rh + 1) * 4, :],
                            in_=src[:, rh * 4:(rh + 1) * 4, :],
                        )

            def emit_outproj(b, ot, dma_eng=None):
                """Out-projection columns [ot*128,(ot+1)*128) for batch b."""
                pp = mmps.tile([128, NW], F32, tag="mm", name="pp_out")
                for ht in range(ET):
                    nc.tensor.matmul(
                        pp,
                        wout_sb[:, ht, ot * 128:(ot + 1) * 128],
                        g_sb[b][:, ht, :],
                        start=(ht == 0),
                        stop=(ht == ET - 1),
                    )
                ob = vt_pool.tile([128, NW], F32, tag="ob", name="ob")
                nc.vector.tensor_scalar_add(ob, pp, bout_sb[:, ot:ot + 1])
                # output stays column-major [o, t]; the host transposes
                # during unshard — keeps 32 PE transposes + copies out of
                # the clock-throttled tail
                (dma_eng or nc.sync).dma_start(
                    out=out_ap[ot * 128:(ot + 1) * 128, b * NW:(b + 1) * NW],
                    in_=ob,
                )

            def emit_outproj_halves(b, ot):
                """emit_outproj split into two filler closures (see
                emit_proj_halves for why)."""
                st = {}

                def half_a():
                    st["pp"] = mmps.tile([128, NW], F32, tag="mm", name="pp_out")
                    for ht in range(ET // 2):
                        nc.tensor.matmul(
                            st["pp"],
                            wout_sb[:, ht, ot * 128:(ot + 1) * 128],
                            g_sb[b][:, ht, :],
                            start=(ht == 0),
                            stop=False,
                        )

                def half_b():
                    pp = st["pp"]
                    for ht in range(ET // 2, ET):
                        nc.tensor.matmul(
                            pp,
                            wout_sb[:, ht, ot * 128:(ot + 1) * 128],
                            g_sb[b][:, ht, :],
                            start=False,
                            stop=(ht == ET - 1),
                        )
                    ob = vt_pool.tile([128, NW], F32, tag="ob", name="ob")
                    nc.vector.tensor_scalar_add(ob, pp, bout_sb[:, ot:ot + 1])
                    nc.sync.dma_start(
                        out=out_ap[ot * 128:(ot + 1) * 128,
                                   b * NW:(b + 1) * NW],
                        in_=ob,
                    )

                return half_a, half_b

            def emit_outproj_p1(b, ot):
                """Lower contraction half of an out-proj column block
                (head-0 features, partitions 0-63 — gathered by the EARLY
                per-batch A2A): real PE fill during the last A2A wait, so
                the clock stays up and only the upper half remains after
                the final gather. Partial saved to SBUF so PSUM rotates."""
                pp = mmps.tile([128, NW], F32, tag="mm", name="pp_h1")
                for ht in range(ET):
                    nc.tensor.matmul(
                        pp,
                        wout_sb[0:64, ht, ot * 128:(ot + 1) * 128],
                        g_sb[b][0:64, ht, :],
                        start=(ht == 0),
                        stop=(ht == ET - 1),
                    )
                sv = saved_pool.tile([128, NW], F32, tag="sv", name="sv")
                nc.vector.tensor_copy(sv, pp)
                return sv

            def emit_outproj_p2(b, ot, sv, dma_eng=None):
                pp = mmps.tile([128, NW], F32, tag="mm", name="pp_h2")
                for ht in range(ET):
                    nc.tensor.matmul(
                        pp,
                        wout_sb[64:128, ht, ot * 128:(ot + 1) * 128],
                        g_sb[b][64:128, ht, :],
                        start=(ht == 0),
                        stop=(ht == ET - 1),
                    )
                ob = vt_pool.tile([128, NW], F32, tag="ob", name="ob")
                # ob = (upper_half + bias) + saved_lower_half, one DVE pass
                nc.vector.scalar_tensor_tensor(
                    ob, pp, bout_sb[:, ot:ot + 1], sv,
                    mybir.AluOpType.add, mybir.AluOpType.add,
                )
                # half-partition DMA split: these are the last transfers
                # before kernel close — two 64-descriptor DMAs drain on two
                # queues in half the time of one 128-descriptor DMA
                for ph in range(2):
                    (dma_eng or nc.sync).dma_start(
                        out=out_ap[ot * 128 + ph * 64:ot * 128 + (ph + 1) * 64,
                                   b * NW:(b + 1) * NW],
                        in_=ob[ph * 64:(ph + 1) * 64, :],
                    )

            # ---- emission schedule: software-pipelined. fin(i) lands one
            # attention iteration after att(i) so the reciprocal is ready
            # before its broadcast matmul enters the PE stream. ----
            ATT0 = [(0, h, tqh) for h in range(HPC) for tqh in range(2)]
            ATT1 = [(1, h, tqh) for h in range(HPC) for tqh in range(2)]
            pend = {}

            # batch 0: the first attention iteration starts as soon as the
            # chunks covering its tk-tiles land; later iterations weave
            # chunk/finish/out-proj pieces between tk-groups so neither the
            # PE nor ScalarE sits idle at iteration boundaries
            # chunk DMAs are prefetched one weave-slot ahead of their
            # projections so the projection stream never waits on HBM,
            # without front-loading the whole x transfer into one burst
            emit_chunk_dma(0, spread=True)
            # minimal projection prefix before attention starts: the first
            # 4 score tiles need only q/k of chunk 0, q of chunk 1 (for the
            # row-swapped qt2 half) and v of chunk 0 — k/v of chunk 1 can
            # project while the first EXPs stream
            emit_proj(0, "q")
            emit_proj(0, "k")
            emit_proj(1, "q")
            emit_proj(0, "v")
            A0 = AttIter(*ATT0[0])
            A0.advance(4)
            emit_chunk_dma(1)
            emit_proj(1, "k")
            emit_proj(1, "v")
            A0.advance(8)
            emit_projs(2)
            A0.advance(12)
            emit_chunk_dma(2)
            emit_projs(3)
            pend[0] = A0.finish()
            # remaining projection / out-proj pieces are woven INSIDE later
            # iterations as HALF-pieces: a block between two iterations
            # stalls the EXP stream for its whole duration, and even a
            # whole piece (~1.8us) drains the 2-tile scores lookahead.
            # Chunk 4-7 k/v land just ahead of the score/PV tiles that
            # consume them (batch-1 tk tiles 8-15 read chunks 6-7).
            p4q = emit_proj_halves(4, "q")
            p4k = emit_proj_halves(4, "k")
            p4v = emit_proj_halves(4, "v")
            pend[1] = weave(*ATT0[1], [
                p4q[0], p4q[1], p4k[0], p4k[1],
                lambda: emit_finish(*ATT0[0], *pend[0]),
                p4v[0], p4v[1],
            ])
            emit_chunk_dma(3)
            p5q = emit_proj_halves(5, "q")
            p5k = emit_proj_halves(5, "k")
            p5v = emit_proj_halves(5, "v")
            pend[2] = weave(*ATT0[2], [
                p5q[0], p5q[1], p5k[0], p5k[1],
                lambda: emit_finish(*ATT0[1], *pend[1]),
                p5v[0], p5v[1],
            ])
            emit_a2a(0, 0)
            # 2MB weight load split across two queues: 1024 descriptors
            # on one queue is ~43us of drain — too close to its first
            # consumer under co-tenant DMA pressure
            wout_view = wout_ap.rearrange("(ht p) o -> p ht o", p=128)
            nc.gpsimd.dma_start(out=wout_sb[:, 0:4, :], in_=wout_view[:, 0:4, :])
            nc.sync.dma_start(out=wout_sb[:, 4:8, :], in_=wout_view[:, 4:8, :])
            p6k = emit_proj_halves(6, "k")
            p6v = emit_proj_halves(6, "v")
            p6q = emit_proj_halves(6, "q")
            pend[3] = weave(*ATT0[3], [
                p6k[0], p6k[1],
                lambda: emit_finish(*ATT0[2], *pend[2]),
                p6v[0], p6v[1], p6q[0], p6q[1],
            ])
            # batch 1 attention, interleaved with batch-0 out-projection
            p7k = emit_proj_halves(7, "k")
            p7v = emit_proj_halves(7, "v")
            p7q = emit_proj_halves(7, "q")
            pend[4] = weave(*ATT1[0], [
                p7k[0], p7k[1],
                lambda: emit_finish(*ATT0[3], *pend[3]),
                p7v[0], p7v[1], p7q[0], p7q[1],
            ])
            emit_a2a(0, 1)
            pend[5] = weave(*ATT1[1], [
                lambda: emit_finish(*ATT1[0], *pend[4]),
                lambda: (emit_gather(0, 0), emit_gather(0, 1)),
            ])
            emit_outproj(0, 0)
            emit_outproj(0, 1)
            pend[6] = weave(*ATT1[2], [
                lambda: (emit_finish(*ATT1[1], *pend[5]), emit_a2a(1, 0)),
            ])
            emit_outproj(0, 2)
            # last attention iteration: weave the previous finish INTO it so
            # its DVE/PE work runs during the iteration and the final
            # finish -> A2A trigger chain stays short. Everything that
            # depends on a PEER (gathers) is emitted strictly AFTER the last
            # A2A trigger: under cross-core skew, a peer-gated DMA emitted
            # earlier stalls the sync queue / PE stream and compounds the
            # skew into the critical path.
            emit_outproj(0, 3)
            pend[7] = weave(
                *ATT1[3],
                [lambda: emit_finish(*ATT1[2], *pend[6])],
                scalar_copy=True,
            )
            emit_outproj(0, 4)
            # keep the PE busy through the ~6us reciprocal DMA round-trip
            # that gates the final normalization: an idle PE drops to
            # 1.2GHz and everything after (broadcasts, batch-0 out-proj,
            # the p1 half-chains) would run at half rate. Fine-grained
            # fillers keep the overshoot past recip-ready under ~0.2us.
            for _ in range(40):
                warm = mmps.tile([128, NW], F32, tag="mm", name="warm")
                nc.tensor.matmul(warm, identb, wout_sb[:, 0, 0:NW])
            emit_finish(*ATT1[3], *pend[7])
            emit_a2a(1, 1)
            # fill the exposed A2A wait: remaining batch-0 columns, then
            # the lower contraction halves of batch-1's out-projections
            # (their head-0 inputs arrived with the earlier a2a(1,0))
            emit_outproj(0, 5)
            emit_outproj(0, 6)
            emit_outproj(0, 7)
            emit_gather(1, 0, split=True)
            svs = [emit_outproj_p1(1, ot) for ot in range(ET)]
            emit_gather(1, 1, split=True)
            # bridge the short window between the last local matmul and the
            # final gather landing (~3us in lockstep): ANY idle resets the
            # PE p-state to 1.2GHz, which would double the cost of the
            # 64 upper-half matmuls that follow
            for _ in range(14):
                warm = mmps.tile([128, NW], F32, tag="mm", name="warm")
                nc.tensor.matmul(warm, identb, wout_sb[:, 0, 0:NW])
            # alternate output-DMA queues in the tail: 8 serial descriptor
            # generations on the sync sequencer would add ~4us after the
            # last bias-add
            for ot in range(ET):
                emit_outproj_p2(
                    1, ot, svs[ot], dma_eng=(nc.scalar if ot % 2 else nc.sync)
                )
    nc.compile()
    return nc


def shard_inputs(x, w_qkv, b_qkv, w_out, b_out):
    """Split full inputs into the 8 per-core input maps (bf16 compute).
    x is transposed host-side so projections need no on-device transpose."""
    x2d = np.asarray(x, dtype=np.float32).reshape(T, D)
    xt = np.ascontiguousarray(x2d.T.astype(BF))  # [D, T]
    w_qkv = np.asarray(w_qkv, dtype=np.float32)
    b_qkv = np.asarray(b_qkv, dtype=np.float32)
    w_out = np.ascontiguousarray(np.asarray(w_out, dtype=np.float32).astype(BF))
    b_out = np.asarray(b_out, dtype=np.float32)
    bout_r = np.ascontiguousarray(b_out.reshape(ET, 128).T)  # [p, ot]
    def rearr(w):
        # [D, CW] -> [128, ET*CW]: row p holds the ET contraction tiles
        # back-to-back so the device-side DMA is line-contiguous
        return np.ascontiguousarray(
            w.reshape(ET, 128, CW).transpose(1, 0, 2).reshape(128, ET * CW)
        )

    in_maps = []
    for i in range(N_CORES):
        c0 = i * CW
        wq = rearr(w_qkv[:, c0:c0 + CW] * SCALE).astype(BF)
        wk = rearr(w_qkv[:, D + c0:D + c0 + CW]).astype(BF)
        wv = rearr(w_qkv[:, 2 * D + c0:2 * D + c0 + CW]).astype(BF)
        bq = (b_qkv[c0:c0 + CW] * SCALE).reshape(CW, 1)
        bk = b_qkv[D + c0:D + c0 + CW].reshape(CW, 1)
        bv = b_qkv[2 * D + c0:2 * D + c0 + CW].reshape(CW, 1)
        in_maps.append({
            "xt": xt,
            "wq": wq, "wk": wk, "wv": wv,
            "bq": np.ascontiguousarray(bq),
            "bk": np.ascontiguousarray(bk),
            "bv": np.ascontiguousarray(bv),
            "wout": w_out,
            "bout": bout_r,
        })
    return in_maps


def get_nc():
    global _CACHED_NC
    if _CACHED_NC is None:
        _CACHED_NC = build()
    return _CACHED_NC


def run(in_maps, trace=False, **kw):
    nc = get_nc()
    return run_bass_kernel_spmd(
        nc, in_maps, core_ids=list(range(N_CORES)), trace=trace, **kw
    )


def assemble(results):
    """Each core returns [1024, 512] column-major: cols 0..255 = its
    256-token slice of batch 0, cols 256..511 = its slice of batch 1;
    transposed to row-major here during unshard."""
    out = np.empty((T, D), dtype=np.float32)
    for i, r in enumerate(results):
        o = r["out"]
        out[i * NW:(i + 1) * NW] = o[:, :NW].T
        out[S + i * NW:S + (i + 1) * NW] = o[:, NW:].T
    return out.reshape(B, S, D)


def kernel(x, w_qkv, b_qkv, w_out, b_out):
    in_maps = shard_inputs(x, w_qkv, b_qkv, w_out, b_out)
    res = run(in_maps, trace=False)
    return assemble(res.results)

